# revision 5
# baseline (speedup 1.0000x reference)
"""Trainium2 Bass kernel for nn_MixUniformAffineQuantizer.

kernel(x, upbound_factor, lowbound_factor) -> [4096, 11008] f32.

Rows sharded 512/core across 8 NeuronCores (row-parallel, no
communication). Per core, 4 row-tiles x 2 col-chunks of [128 x 5504].

Engine split (per chunk):
  - DVE: per-group min/max tensor_reduce (DVE-only capability), the
    reciprocals, and the fused dequant stt on most chunks
  - ACT: per-group round v = x*(1/s) + M (Identity activation, bias
    M = 1.5*2^23 lands the sum on the fp32 integer grid = round-half-even,
    matching XLA bitwise)
  - Pool (gpsimd): per-group clamp in the magic-offset domain
    v <- min(max(v, M - z), M + qmax - z) via tensor_scalar with two
    per-group [128,1] scalar APs, plus the per-row-tile stat math
  - dequant: y = (v - M) * s in one scalar_tensor_tensor with a stride-0
    broadcast scale view; output written as bf16 (halves output DMA; abs
    err <= 2^-9 * |y| ~ 0.011 << the 2e-2 rel tolerance)

Clamping against B = fl(M + (qmax - z)) rounds B onto the integer grid
(ulp=1 near M), so upper-clipped elements differ from the reference's
non-integer qmax (e.g. 14.999921) by <= 8e-5 * s -- far below tolerance;
all other elements match the f32 reference exactly before the bf16 store.

sigmoid(upbound/lowbound) is computed host-side with jax (default
device), matching the reference bitwise.
"""
import sys
import numpy as np

for _p in ("/opt/trn_rl_repo", "/root/.axon_site/_ro/trn_rl_repo"):
    if _p not in sys.path:
        sys.path.append(_p)

from contextlib import ExitStack
import concourse.bass as bass
import concourse.tile as tile
from concourse import bacc, mybir
from concourse.bass_utils import run_bass_kernel_spmd

F32 = mybir.dt.float32
BF16 = mybir.dt.bfloat16
ALU = mybir.AluOpType
ACTF = mybir.ActivationFunctionType

ROWS, COLS, G, NB = 4096, 11008, 128, 86
NCORES = 8
R = ROWS // NCORES    # 512 rows per core
NT = R // 128         # 4 row-tiles
NCH = 2               # col chunks per row-tile
GCH = NB // NCH       # 43 groups per chunk
CH = GCH * G          # 5504 cols per chunk
M = 12582912.0        # 1.5*2^23 round-to-even magic
CLIPMIN, CLIPMAX = 1e-5, 1e4

_PREC = np.array([1] + [2, 3, 4, 3, 2] * 17, dtype=np.int32)

LOOKAHEAD = 2
CLAMP_ENG = "PPPPPPPP"   # per-chunk: P=Pool, D=DVE
DEQ_ENG = "DPDDDPDD"     # 6 DVE (stt) / 2 Pool (per-group TS)
STATS_ENG = "P"          # stat math engine (reciprocals always DVE)
OUT_BF16 = True

_LEVELS = None


def _levels_jax():
    """2^p - 1 exactly as the jax reference computes it (default device)."""
    global _LEVELS
    if _LEVELS is None:
        import jax.numpy as jnp
        _LEVELS = np.asarray(
            jnp.exp2(jnp.asarray(_PREC).astype(jnp.float32)) - 1.0
        ).astype(np.float32)
    return _LEVELS


def _bv(small_ap, width=G):
    """[128, n] AP -> [128, n, width] stride-0 broadcast view."""
    return bass.AP(small_ap.tensor, small_ap.offset,
                   [small_ap.ap[0], small_ap.ap[1], [0, width]])


def _build(nc):
    lvj = _levels_jax()
    q1 = float(lvj[0])  # ternary clip-high (~0.99999833)
    ODT = BF16 if OUT_BF16 else F32
    x = nc.dram_tensor("x", [R, COLS], F32, kind="ExternalInput").ap()
    su = nc.dram_tensor("su", [R, NB], F32, kind="ExternalInput").ap()
    sl = nc.dram_tensor("sl", [R, NB], F32, kind="ExternalInput").ap()
    su05 = nc.dram_tensor("su05", [R, 1], F32, kind="ExternalInput").ap()
    ilv = nc.dram_tensor("inv_levels", [128, NB], F32, kind="ExternalInput").ap()
    lvM = nc.dram_tensor("lvM", [128, NB], F32, kind="ExternalInput").ap()
    out = nc.dram_tensor("out", [R, COLS], ODT, kind="ExternalOutput").ap()

    with tile.TileContext(nc) as tc, ExitStack() as ctx:
        cpool = ctx.enter_context(tc.tile_pool(name="const", bufs=1))
        xpool = ctx.enter_context(tc.tile_pool(name="xp", bufs=LOOKAHEAD + 2))
        vpool = ctx.enter_context(tc.tile_pool(name="vp", bufs=2))
        ypool = ctx.enter_context(tc.tile_pool(name="yp", bufs=2))
        rpool = ctx.enter_context(tc.tile_pool(name="rowp", bufs=2))
        spool = ctx.enter_context(tc.tile_pool(name="statp", bufs=2))
        gpool = ctx.enter_context(tc.tile_pool(name="gp", bufs=4))

        lvM_t = cpool.tile([128, NB], F32, tag="lvM")
        nc.sync.dma_start(lvM_t[:], lvM[:])
        ilv_t = cpool.tile([128, NB], F32, tag="ilv")
        nc.sync.dma_start(ilv_t[:], ilv[:])
        Mb = cpool.tile([128, 1], F32, tag="Mb")
        nc.vector.memset(Mb[:], M)

        SE = nc.gpsimd if STATS_ENG == "P" else nc.vector

        chunks = [(rt, c) for rt in range(NT) for c in range(NCH)]
        n = len(chunks)
        xstate = {}
        rowstate = {}

        def front(k):
            rt, c = chunks[k]
            if c == 0:
                rs_ = {}
                rs_["su"] = rpool.tile([128, NB], F32, tag="su", name="su_t")
                nc.sync.dma_start(rs_["su"][:], su[rt * 128:(rt + 1) * 128, :])
                rs_["sl"] = rpool.tile([128, NB], F32, tag="sl", name="sl_t")
                nc.sync.dma_start(rs_["sl"][:], sl[rt * 128:(rt + 1) * 128, :])
                rs_["s5"] = rpool.tile([128, 1], F32, tag="su05", name="su05_t")
                nc.sync.dma_start(rs_["s5"][:], su05[rt * 128:(rt + 1) * 128, :])
                rs_["rmin"] = spool.tile([128, NB], F32, tag="rmin", name="rmin_t")
                rs_["rmax"] = spool.tile([128, NB], F32, tag="rmax", name="rmax_t")
                rowstate[rt] = rs_
            rs_ = rowstate[rt]

            xt = xpool.tile([128, CH], F32, tag="x")
            for q in range(4):
                nc.sync.dma_start(
                    xt[q * 32:(q + 1) * 32, :],
                    x[rt * 128 + q * 32:rt * 128 + (q + 1) * 32,
                      c * CH:(c + 1) * CH])
            xstate[k] = xt

            gsl = slice(c * GCH, (c + 1) * GCH)
            xv = xt[:, :].rearrange("p (g j) -> p g j", j=G)
            nc.vector.tensor_reduce(rs_["rmin"][:, gsl], xv,
                                    axis=mybir.AxisListType.X, op=ALU.min)
            nc.vector.tensor_reduce(rs_["rmax"][:, gsl], xv,
                                    axis=mybir.AxisListType.X, op=ALU.max)
            if c == 0:
                x0v = xt[:, 0:G].rearrange("p (g j) -> p g j", j=G)
                rs_["rsum"] = spool.tile([128, 1], F32, tag="rsum", name="rsum_t")
                nc.vector.tensor_reduce(rs_["rsum"][:], x0v,
                                        axis=mybir.AxisListType.X, op=ALU.add)
                rs_["rabs"] = spool.tile([128, 1], F32, tag="rabs", name="rabs_t")
                nc.vector.tensor_reduce(rs_["rabs"][:], x0v,
                                        axis=mybir.AxisListType.X, op=ALU.add,
                                        apply_absolute_value=True)

        def stats(rt):
            rs_ = rowstate[rt]

            def t(tag):
                return spool.tile([128, NB], F32, tag=tag, name=tag + "_t")

            xsmax = t("xsmax")
            SE.tensor_tensor(xsmax[:], rs_["su"][:], rs_["rmax"][:], op=ALU.mult)
            xsmin = t("xsmin")
            SE.tensor_tensor(xsmin[:], rs_["sl"][:], rs_["rmin"][:], op=ALU.mult)
            diff = t("diff")
            SE.tensor_tensor(diff[:], xsmax[:], xsmin[:], op=ALU.subtract)
            scale_r = t("scale_r")
            SE.tensor_tensor(scale_r[:], diff[:], ilv_t[:], op=ALU.mult)
            rcp = t("rcp")
            nc.vector.reciprocal(rcp[:], scale_r[:])
            t1 = t("t1")
            SE.tensor_tensor(t1[:], xsmin[:], rcp[:], op=ALU.mult)
            t2 = t("t2")
            SE.tensor_scalar(t2[:], t1[:], -CLIPMAX, CLIPMAX, op0=ALU.max, op1=ALU.min)
            t3 = t("t3")
            SE.tensor_scalar(t3[:], t2[:], M, M, op0=ALU.add, op1=ALU.subtract)
            scl = t("scl")
            SE.tensor_scalar(scl[:], scale_r[:], CLIPMIN, CLIPMAX, op0=ALU.max, op1=ALU.min)
            rs = t("rs")
            nc.vector.reciprocal(rs[:], scl[:])
            Mnz = t("Mnz")
            SE.tensor_scalar(Mnz[:], t3[:], M, None, op0=ALU.add)
            MQZ = t("MQZ")
            SE.tensor_tensor(MQZ[:], t3[:], lvM_t[:], op=ALU.add)
            rs_.update(scl=scl, rs=rs, Mnz=Mnz, MQZ=MQZ)

            # ternary group stats ([128,1] -- tiny)
            nzt_a = spool.tile([128, 1], F32, tag="nzt_a")
            SE.tensor_scalar(nzt_a[:], rs_["rsum"][:], -1.0 / 128.0, -CLIPMAX,
                             op0=ALU.mult, op1=ALU.max)
            nzt = spool.tile([128, 1], F32, tag="nzt")
            SE.tensor_scalar(nzt[:], nzt_a[:], CLIPMAX, None, op0=ALU.min)
            sta = spool.tile([128, 1], F32, tag="sta")
            SE.tensor_scalar(sta[:], rs_["rabs"][:], 1.0 / 128.0, rs_["s5"][:],
                             op0=ALU.mult, op1=ALU.mult)
            stt = spool.tile([128, 1], F32, tag="stt")
            SE.tensor_scalar(stt[:], sta[:], CLIPMIN, CLIPMAX,
                             op0=ALU.max, op1=ALU.min)
            rs_.update(nzt=nzt, stt=stt)

        def back(j):
            rt, c = chunks[j]
            if c == 0:
                stats(rt)
            rs_ = rowstate[rt]
            xt = xstate.pop(j)
            g0 = 1 if c == 0 else 0
            cg = c * GCH

            vt = vpool.tile([128, CH], F32, tag="v")
            for g in range(g0, GCH):
                nc.scalar.activation(vt[:, g * G:(g + 1) * G],
                                     xt[:, g * G:(g + 1) * G], ACTF.Identity,
                                     bias=Mb[:], scale=rs_["rs"][:, cg + g:cg + g + 1])

            ce = nc.gpsimd if CLAMP_ENG[j] == "P" else nc.vector
            # reversed: first clamp waits on the whole ACT loop; Tile can
            # then prove the remaining deps satisfied by program order
            for g in reversed(range(g0, GCH)):
                sl_ = slice(g * G, (g + 1) * G)
                ce.tensor_scalar(vt[:, sl_], vt[:, sl_],
                                 rs_["Mnz"][:, cg + g:cg + g + 1],
                                 rs_["MQZ"][:, cg + g:cg + g + 1],
                                 op0=ALU.max, op1=ALU.min)

            ODT = BF16 if OUT_BF16 else F32
            yt = ypool.tile([128, CH], ODT, tag="y")
            if DEQ_ENG[j] == "P":
                # STT is not a valid Pool opcode on v3; per-group TS instead.
                # Same engine as the clamp loop -> program-order, no sems.
                for g in range(g0, GCH):
                    sl_ = slice(g * G, (g + 1) * G)
                    nc.gpsimd.tensor_scalar(yt[:, sl_], vt[:, sl_], M,
                                            rs_["scl"][:, cg + g:cg + g + 1],
                                            op0=ALU.subtract, op1=ALU.mult)
            else:
                off = g0 * G
                vv = vt[:, off:CH].rearrange("p (g j) -> p g j", j=G)
                yv = yt[:, off:CH].rearrange("p (g j) -> p g j", j=G)
                nc.vector.scalar_tensor_tensor(
                    yv, vv, M, _bv(rs_["scl"][:, cg + g0:cg + GCH]),
                    op0=ALU.subtract, op1=ALU.mult)

            if c == 0:
                v0 = gpool.tile([128, G], F32, tag="v0")
                nc.scalar.sign(v0[:], xt[:, 0:G], bias=rs_["nzt"][:])
                nc.vector.tensor_scalar(yt[:, 0:G], v0[:], q1, rs_["stt"][:],
                                        op0=ALU.min, op1=ALU.mult)

            nc.sync.dma_start(out[rt * 128:(rt + 1) * 128, c * CH:(c + 1) * CH],
                              yt[:])

        for k in range(n + LOOKAHEAD):
            if k < n:
                front(k)
            if k >= LOOKAHEAD:
                back(k - LOOKAHEAD)
    return nc


_COMPILED = None


def _get_compiled():
    global _COMPILED
    if _COMPILED is None:
        nc = bacc.Bacc("TRN2", target_bir_lowering=False, debug=False)
        _build(nc)
        nc.compile()
        _COMPILED = nc
    return _COMPILED


def prepare_in_maps(x, upbound_factor, lowbound_factor):
    import jax, jax.numpy as jnp
    x = np.ascontiguousarray(np.asarray(x, dtype=np.float32))
    up = np.asarray(upbound_factor, dtype=np.float32)
    low = np.asarray(lowbound_factor, dtype=np.float32)
    assert x.shape == (ROWS, COLS) and up.shape == (ROWS, NB) and low.shape == (ROWS, NB)

    # host precompute (matches the reference's own jax ops bitwise)
    su = np.asarray(jax.nn.sigmoid(jnp.asarray(up))).astype(np.float32)
    sl = np.asarray(jax.nn.sigmoid(jnp.asarray(low))).astype(np.float32)
    su05 = (su[:, 0:1] + np.float32(0.5)).astype(np.float32)
    lvj = _levels_jax()
    ilv = np.ascontiguousarray(
        np.broadcast_to((np.float32(1.0) / lvj)[None, :], (128, NB)), dtype=np.float32)
    lvM = np.ascontiguousarray(
        np.broadcast_to((np.float32(M) + lvj)[None, :], (128, NB)), dtype=np.float32)

    in_maps = []
    for i in range(NCORES):
        r0, r1 = i * R, (i + 1) * R
        in_maps.append({
            "x": np.ascontiguousarray(x[r0:r1]),
            "su": np.ascontiguousarray(su[r0:r1]),
            "sl": np.ascontiguousarray(sl[r0:r1]),
            "su05": np.ascontiguousarray(su05[r0:r1]),
            "inv_levels": ilv,
            "lvM": lvM,
        })
    return in_maps


def kernel(x, upbound_factor, lowbound_factor):
    in_maps = prepare_in_maps(x, upbound_factor, lowbound_factor)
    nc = _get_compiled()
    res = run_bass_kernel_spmd(nc, in_maps, core_ids=list(range(NCORES)), trace=False)
    return np.concatenate([np.asarray(res.results[i]["out"]).astype(np.float32)
                           for i in range(NCORES)], axis=0)


# revision 8
# speedup vs baseline: 3.8490x; 3.8490x over previous
"""Trainium2 Bass kernel for nn_MixUniformAffineQuantizer.

kernel(x, upbound_factor, lowbound_factor) -> [4096, 11008] f32.

Rows sharded 512/core across 8 NeuronCores (row-parallel, no
communication). Per core, 4 row-tiles x 2 col-chunks of [128 x 5504].

Engine split (per chunk):
  - DVE: per-group min/max tensor_reduce (DVE-only capability), the
    reciprocals, and the fused dequant stt on most chunks
  - ACT: per-group round v = x*(1/s) + M (Identity activation, bias
    M = 1.5*2^23 lands the sum on the fp32 integer grid = round-half-even,
    matching XLA bitwise)
  - Pool (gpsimd): per-group clamp in the magic-offset domain
    v <- min(max(v, M - z), M + qmax - z) via tensor_scalar with two
    per-group [128,1] scalar APs, plus the per-row-tile stat math
  - dequant: y = (v - M) * s in one scalar_tensor_tensor with a stride-0
    broadcast scale view; output written as bf16 (halves output DMA; abs
    err <= 2^-9 * |y| ~ 0.011 << the 2e-2 rel tolerance)

Clamping against B = fl(M + (qmax - z)) rounds B onto the integer grid
(ulp=1 near M), so upper-clipped elements differ from the reference's
non-integer qmax (e.g. 14.999921) by <= 8e-5 * s -- far below tolerance;
all other elements match the f32 reference exactly before the bf16 store.

sigmoid(upbound/lowbound) is computed host-side with jax (default
device), matching the reference bitwise.
"""
import sys
import numpy as np

for _p in ("/opt/trn_rl_repo", "/root/.axon_site/_ro/trn_rl_repo"):
    if _p not in sys.path:
        sys.path.append(_p)

from contextlib import ExitStack
import concourse.bass as bass
import concourse.tile as tile
from concourse import bacc, mybir
from concourse.bass_utils import run_bass_kernel_spmd
from concourse import dve_ops
from concourse.dve_ops import DveOp
from concourse.dve_spec import (
    Spec, Src0, C0, C1, C2, C3, maxx, minn, _spill_c3_to_src1, lower,
    _has_src1 as _has_src1,
)
from concourse.dve_uop import DveOpSpec

F32 = mybir.dt.float32
BF16 = mybir.dt.bfloat16
ALU = mybir.AluOpType
ACTF = mybir.ActivationFunctionType


def _make_clamp_deq():
    """Register the fused per-group tail op with the custom-DVE registry:
    out = (min(max(in0, s0), s1) - imm2) * in1   (in1 = latched C3 scalar).
    One DVE pass replaces clamp (tensor_scalar) + dequant (stt)."""
    name = "CLAMP_DEQ_ANT"
    if name in dve_ops._SUB_OPCODE_FOR_NAME:
        return next(op for op in dve_ops.OPS if op.name == name)
    body = _spill_c3_to_src1((minn(maxx(Src0, C0), C1) - C2) * C3)
    spec = Spec(
        body=body,
        reference=lambda in0, in1, s0, s1, imm2:
            (np.minimum(np.maximum(in0, s0), s1) - imm2) * in1,
    )
    row = dve_ops._CUSTOM_DVE_ROW_BASE + len(dve_ops.OPS)
    assert row < 0x20
    shas = {}
    for ver in ("v3", "v4"):
        try:
            tmp = DveOpSpec(name=name, opcode=row, uops=lower(spec, ver=ver),
                            rd1_en=_has_src1(spec))
            shas[ver] = tmp.sha(ver)
        except Exception:
            pass
    op = DveOp(name, spec, subdim=False, uops_sha=shas)
    dve_ops.OPS.append(op)
    dve_ops.CUSTOM_DVE_SPECS[name] = spec
    dve_ops._SUB_OPCODE_FOR_NAME[name] = row
    return op


CLAMP_DEQ = _make_clamp_deq()

ROWS, COLS, G, NB = 4096, 11008, 128, 86
NCORES = 8
R = ROWS // NCORES    # 512 rows per core
NT = R // 128         # 4 row-tiles
NCH = 2               # col chunks per row-tile
GCH = NB // NCH       # 43 groups per chunk
CH = GCH * G          # 5504 cols per chunk
M = 12582912.0        # 1.5*2^23 round-to-even magic
CLIPMIN, CLIPMAX = 1e-5, 1e4

_PREC = np.array([1] + [2, 3, 4, 3, 2] * 17, dtype=np.int32)

LOOKAHEAD = 2
STATS_ENG = "P"          # stat math engine (reciprocals always DVE)
OUT_BF16 = True

_LEVELS = None


def _levels_jax():
    """2^p - 1 exactly as the jax reference computes it (default device)."""
    global _LEVELS
    if _LEVELS is None:
        import jax.numpy as jnp
        _LEVELS = np.asarray(
            jnp.exp2(jnp.asarray(_PREC).astype(jnp.float32)) - 1.0
        ).astype(np.float32)
    return _LEVELS


def _bv(small_ap, width=G):
    """[128, n] AP -> [128, n, width] stride-0 broadcast view."""
    return bass.AP(small_ap.tensor, small_ap.offset,
                   [small_ap.ap[0], small_ap.ap[1], [0, width]])


def _build(nc):
    lvj = _levels_jax()
    q1 = float(lvj[0])  # ternary clip-high (~0.99999833)
    ODT = BF16 if OUT_BF16 else F32
    x = nc.dram_tensor("x", [R, COLS], F32, kind="ExternalInput").ap()
    su = nc.dram_tensor("su", [R, NB], F32, kind="ExternalInput").ap()
    sl = nc.dram_tensor("sl", [R, NB], F32, kind="ExternalInput").ap()
    su05 = nc.dram_tensor("su05", [R, 1], F32, kind="ExternalInput").ap()
    ilv = nc.dram_tensor("inv_levels", [128, NB], F32, kind="ExternalInput").ap()
    lvM = nc.dram_tensor("lvM", [128, NB], F32, kind="ExternalInput").ap()
    out = nc.dram_tensor("out", [R, COLS], ODT, kind="ExternalOutput").ap()

    with tile.TileContext(nc) as tc, ExitStack() as ctx:
        cpool = ctx.enter_context(tc.tile_pool(name="const", bufs=1))
        xpool = ctx.enter_context(tc.tile_pool(name="xp", bufs=LOOKAHEAD + 2))
        vpool = ctx.enter_context(tc.tile_pool(name="vp", bufs=2))
        ypool = ctx.enter_context(tc.tile_pool(name="yp", bufs=2))
        rpool = ctx.enter_context(tc.tile_pool(name="rowp", bufs=2))
        spool = ctx.enter_context(tc.tile_pool(name="statp", bufs=2))
        gpool = ctx.enter_context(tc.tile_pool(name="gp", bufs=4))

        lvM_t = cpool.tile([128, NB], F32, tag="lvM")
        nc.sync.dma_start(lvM_t[:], lvM[:])
        ilv_t = cpool.tile([128, NB], F32, tag="ilv")
        nc.sync.dma_start(ilv_t[:], ilv[:])
        Mb = cpool.tile([128, 1], F32, tag="Mb")
        nc.vector.memset(Mb[:], M)

        SE = nc.gpsimd if STATS_ENG == "P" else nc.vector

        chunks = [(rt, c) for rt in range(NT) for c in range(NCH)]
        n = len(chunks)
        xstate = {}
        rowstate = {}

        def front(k):
            rt, c = chunks[k]
            if c == 0:
                rs_ = {}
                rs_["su"] = rpool.tile([128, NB], F32, tag="su", name="su_t")
                nc.sync.dma_start(rs_["su"][:], su[rt * 128:(rt + 1) * 128, :])
                rs_["sl"] = rpool.tile([128, NB], F32, tag="sl", name="sl_t")
                nc.sync.dma_start(rs_["sl"][:], sl[rt * 128:(rt + 1) * 128, :])
                rs_["s5"] = rpool.tile([128, 1], F32, tag="su05", name="su05_t")
                nc.sync.dma_start(rs_["s5"][:], su05[rt * 128:(rt + 1) * 128, :])
                rs_["rmin"] = spool.tile([128, NB], F32, tag="rmin", name="rmin_t")
                rs_["rmax"] = spool.tile([128, NB], F32, tag="rmax", name="rmax_t")
                rowstate[rt] = rs_
            rs_ = rowstate[rt]

            xt = xpool.tile([128, CH], F32, tag="x")
            for q in range(4):
                nc.sync.dma_start(
                    xt[q * 32:(q + 1) * 32, :],
                    x[rt * 128 + q * 32:rt * 128 + (q + 1) * 32,
                      c * CH:(c + 1) * CH])
            xstate[k] = xt

            gsl = slice(c * GCH, (c + 1) * GCH)
            xv = xt[:, :].rearrange("p (g j) -> p g j", j=G)
            nc.vector.tensor_reduce(rs_["rmin"][:, gsl], xv,
                                    axis=mybir.AxisListType.X, op=ALU.min)
            nc.vector.tensor_reduce(rs_["rmax"][:, gsl], xv,
                                    axis=mybir.AxisListType.X, op=ALU.max)
            if c == 0:
                x0v = xt[:, 0:G].rearrange("p (g j) -> p g j", j=G)
                rs_["rsum"] = spool.tile([128, 1], F32, tag="rsum", name="rsum_t")
                nc.vector.tensor_reduce(rs_["rsum"][:], x0v,
                                        axis=mybir.AxisListType.X, op=ALU.add)
                rs_["rabs"] = spool.tile([128, 1], F32, tag="rabs", name="rabs_t")
                nc.vector.tensor_reduce(rs_["rabs"][:], x0v,
                                        axis=mybir.AxisListType.X, op=ALU.add,
                                        apply_absolute_value=True)

        def stats(rt):
            rs_ = rowstate[rt]

            def t(tag):
                return spool.tile([128, NB], F32, tag=tag, name=tag + "_t")

            xsmax = t("xsmax")
            SE.tensor_tensor(xsmax[:], rs_["su"][:], rs_["rmax"][:], op=ALU.mult)
            xsmin = t("xsmin")
            SE.tensor_tensor(xsmin[:], rs_["sl"][:], rs_["rmin"][:], op=ALU.mult)
            diff = t("diff")
            SE.tensor_tensor(diff[:], xsmax[:], xsmin[:], op=ALU.subtract)
            scale_r = t("scale_r")
            SE.tensor_tensor(scale_r[:], diff[:], ilv_t[:], op=ALU.mult)
            rcp = t("rcp")
            nc.vector.reciprocal(rcp[:], scale_r[:])
            t1 = t("t1")
            SE.tensor_tensor(t1[:], xsmin[:], rcp[:], op=ALU.mult)
            t2 = t("t2")
            SE.tensor_scalar(t2[:], t1[:], -CLIPMAX, CLIPMAX, op0=ALU.max, op1=ALU.min)
            t3 = t("t3")
            SE.tensor_scalar(t3[:], t2[:], M, M, op0=ALU.add, op1=ALU.subtract)
            scl = t("scl")
            SE.tensor_scalar(scl[:], scale_r[:], CLIPMIN, CLIPMAX, op0=ALU.max, op1=ALU.min)
            rs = t("rs")
            nc.vector.reciprocal(rs[:], scl[:])
            Mnz = t("Mnz")
            SE.tensor_scalar(Mnz[:], t3[:], M, None, op0=ALU.add)
            MQZ = t("MQZ")
            SE.tensor_tensor(MQZ[:], t3[:], lvM_t[:], op=ALU.add)
            rs_.update(scl=scl, rs=rs, Mnz=Mnz, MQZ=MQZ)

            # ternary group stats ([128,1] -- tiny)
            nzt_a = spool.tile([128, 1], F32, tag="nzt_a")
            SE.tensor_scalar(nzt_a[:], rs_["rsum"][:], -1.0 / 128.0, -CLIPMAX,
                             op0=ALU.mult, op1=ALU.max)
            nzt = spool.tile([128, 1], F32, tag="nzt")
            SE.tensor_scalar(nzt[:], nzt_a[:], CLIPMAX, None, op0=ALU.min)
            sta = spool.tile([128, 1], F32, tag="sta")
            SE.tensor_scalar(sta[:], rs_["rabs"][:], 1.0 / 128.0, rs_["s5"][:],
                             op0=ALU.mult, op1=ALU.mult)
            stt = spool.tile([128, 1], F32, tag="stt")
            SE.tensor_scalar(stt[:], sta[:], CLIPMIN, CLIPMAX,
                             op0=ALU.max, op1=ALU.min)
            rs_.update(nzt=nzt, stt=stt)

        def back(j):
            rt, c = chunks[j]
            if c == 0:
                stats(rt)
            rs_ = rowstate[rt]
            xt = xstate.pop(j)
            g0 = 1 if c == 0 else 0
            cg = c * GCH

            vt = vpool.tile([128, CH], F32, tag="v")
            for g in range(g0, GCH):
                nc.scalar.activation(vt[:, g * G:(g + 1) * G],
                                     xt[:, g * G:(g + 1) * G], ACTF.Identity,
                                     bias=Mb[:], scale=rs_["rs"][:, cg + g:cg + g + 1])

            ODT = BF16 if OUT_BF16 else F32
            yt = ypool.tile([128, CH], ODT, tag="y")
            # reversed: first op waits on the whole ACT loop; Tile elides the
            # remaining cross-engine waits as satisfied by program order
            for g in reversed(range(g0, GCH)):
                sl_ = slice(g * G, (g + 1) * G)
                nc.vector._custom_dve(
                    CLAMP_DEQ, out=yt[:, sl_], in0=vt[:, sl_],
                    in1=rs_["scl"][:, cg + g:cg + g + 1],
                    s0=rs_["Mnz"][:, cg + g:cg + g + 1],
                    s1=rs_["MQZ"][:, cg + g:cg + g + 1],
                    imm2=M)

            if c == 0:
                v0 = gpool.tile([128, G], F32, tag="v0")
                nc.scalar.sign(v0[:], xt[:, 0:G], bias=rs_["nzt"][:])
                nc.vector.tensor_scalar(yt[:, 0:G], v0[:], q1, rs_["stt"][:],
                                        op0=ALU.min, op1=ALU.mult)

            nc.sync.dma_start(out[rt * 128:(rt + 1) * 128, c * CH:(c + 1) * CH],
                              yt[:])

        for k in range(n + LOOKAHEAD):
            if k < n:
                front(k)
            if k >= LOOKAHEAD:
                back(k - LOOKAHEAD)
    return nc


_COMPILED = None


def _get_compiled():
    global _COMPILED
    if _COMPILED is None:
        nc = bacc.Bacc("TRN2", target_bir_lowering=False, debug=False)
        _build(nc)
        nc.compile()
        _COMPILED = nc
    return _COMPILED


def prepare_in_maps(x, upbound_factor, lowbound_factor):
    import jax, jax.numpy as jnp
    x = np.ascontiguousarray(np.asarray(x, dtype=np.float32))
    up = np.asarray(upbound_factor, dtype=np.float32)
    low = np.asarray(lowbound_factor, dtype=np.float32)
    assert x.shape == (ROWS, COLS) and up.shape == (ROWS, NB) and low.shape == (ROWS, NB)

    # host precompute (matches the reference's own jax ops bitwise)
    su = np.asarray(jax.nn.sigmoid(jnp.asarray(up))).astype(np.float32)
    sl = np.asarray(jax.nn.sigmoid(jnp.asarray(low))).astype(np.float32)
    su05 = (su[:, 0:1] + np.float32(0.5)).astype(np.float32)
    lvj = _levels_jax()
    ilv = np.ascontiguousarray(
        np.broadcast_to((np.float32(1.0) / lvj)[None, :], (128, NB)), dtype=np.float32)
    lvM = np.ascontiguousarray(
        np.broadcast_to((np.float32(M) + lvj)[None, :], (128, NB)), dtype=np.float32)

    in_maps = []
    for i in range(NCORES):
        r0, r1 = i * R, (i + 1) * R
        in_maps.append({
            "x": np.ascontiguousarray(x[r0:r1]),
            "su": np.ascontiguousarray(su[r0:r1]),
            "sl": np.ascontiguousarray(sl[r0:r1]),
            "su05": np.ascontiguousarray(su05[r0:r1]),
            "inv_levels": ilv,
            "lvM": lvM,
        })
    return in_maps


def kernel(x, upbound_factor, lowbound_factor):
    in_maps = prepare_in_maps(x, upbound_factor, lowbound_factor)
    nc = _get_compiled()
    res = run_bass_kernel_spmd(nc, in_maps, core_ids=list(range(NCORES)), trace=False)
    return np.concatenate([np.asarray(res.results[i]["out"]).astype(np.float32)
                           for i in range(NCORES)], axis=0)
